# revision 58
# baseline (speedup 1.0000x reference)
"""MoE layer (8 experts, top-2) on 8 TRN2 NeuronCores — expert parallelism.

Contract: kernel(**inputs) takes FULL inputs, returns FULL output.
Strategy:
  - Host computes the (tiny) gate: logits -> top-2 -> softmax. This is the
    dispatch step of expert parallelism: tokens are gathered per expert.
  - Core e gets expert e's weights and its gathered tokens (padded to a
    fixed capacity C), computes y = relu(x @ w1 + b1) @ w2 scaled by the
    gate prob, via a Bass/Tile kernel using float32r matmuls.
  - Host scatter-adds the two expert contributions per token (combine step)
    plus the (usually zero) b2 term.

Shapes (hardcoded from the problem spec):
  x [2048, 2, 1024], gate_w [1024, 8], gate_b [8],
  w1 [8, 1024, 4096], b1 [8, 4096], w2 [8, 4096, 1024], b2 [8, 1024].
"""
import sys
import numpy as np

for _p in ("/opt/trn_rl_repo", "/root/.axon_site/_ro/trn_rl_repo"):
    if _p not in sys.path:
        sys.path.insert(0, _p)

import concourse.bacc as bacc
import concourse.tile as tile
import concourse.mybir as mybir
from concourse import bass2jax, mybir as _mybir

N_EXPERTS = 8
TOP_K = 2
S, B, D, F = 2048, 2, 1024, 4096
P = 128
FB = 512                # F-block size streamed through SBUF
NB = F // FB            # 8 F-blocks
FC = FB // P            # 4 partition-tiles of F per block
DK = D // P             # 8 contraction tiles for stage 1
DN = D // 512           # 2 output-column chunks for stage 2

_f32 = mybir.dt.float32
_f32r = mybir.dt.float32r

_NC_CACHE: dict = {}
_C_MAX = 1280           # max capacity per pass (SBUF budget bound)
LAST_DEVICE_NS = -1     # wall-clock of the last device dispatch (incl. transfers)
LAST_C = -1


def _c_chunks(C):
    """Split C (multiple of 128) into near-equal chunks <=512 and >=256.

    f32r matmuls run at full rate only for moving dim >=256; chunks >=384
    also keep the implicit LDWEIGHTS (~128 cols) hidden under streaming.
    """
    k = -(-C // 512)
    base = (C // k) // P * P
    sizes = [base] * k
    rem = C - base * k
    i = 0
    while rem > 0:
        sizes[i] += P
        rem -= P
        i = (i + 1) % k
    out, pos = [], 0
    for s in sizes:
        out.append((pos, s))
        pos += s
    return out


def _build(C, passes=1, *, skew=True, psum1_bufs=8, psum2_bufs=0, w_bufs=2, h_bufs=2,
           head="dkmajor"):
    """Trace + compile the per-core SPMD program for capacity C (multiple of 128, >=256).

    passes>1 repeats the whole compute (same output) — used only for
    differential timing of the device kernel.
    skew=True emits stage1 of block fb+1 before stage2 of block fb so the
    PE can keep streaming matmuls while ACT finishes the relus stage2 needs.
    """
    key = (C, passes, skew, psum1_bufs, psum2_bufs, w_bufs, h_bufs, head)
    if key in _NC_CACHE:
        return _NC_CACHE[key]
    TT = C // P
    nc = bacc.Bacc("TRN2", target_bir_lowering=False, debug=False,
                   enable_asserts=False, num_devices=8)
    xgt_d = nc.dram_tensor("xgt", (D, C), _f32r, kind="ExternalInput").ap()
    w1_d = nc.dram_tensor("w1", (D, F), _f32r, kind="ExternalInput").ap()
    b1_d = nc.dram_tensor("b1", (F,), _f32, kind="ExternalInput").ap()
    w2_d = nc.dram_tensor("w2", (F, D), _f32r, kind="ExternalInput").ap()
    probs_d = nc.dram_tensor("probs", (C,), _f32, kind="ExternalInput").ap()
    y_d = nc.dram_tensor("y", (C, D), _f32, kind="ExternalOutput").ap()

    xgt_r = xgt_d.rearrange("(ko ki) c -> ki ko c", ki=P)      # [128, 8, C]
    w1_r = w1_d.rearrange("(ko ki) f -> ki ko f", ki=P)        # [128, 8, F]
    w2_r = w2_d.rearrange("(fo fi) d -> fi fo d", fi=P)        # [128, 32, D]
    y_r = y_d.rearrange("(t p) d -> p t d", p=P)               # [128, TT, D]

    chunks = _c_chunks(C)

    with tile.TileContext(nc) as tc:
        with tc.tile_pool(name="const", bufs=1) as cpool, \
             tc.tile_pool(name="w1p", bufs=w_bufs) as w1pool, \
             tc.tile_pool(name="w2p", bufs=w_bufs) as w2pool, \
             tc.tile_pool(name="hp", bufs=h_bufs) as hpool, \
             tc.tile_pool(name="ps1", bufs=psum1_bufs, space="PSUM") as psum1, \
             tc.tile_pool(name="ps2", bufs=max(psum2_bufs, 1), space="PSUM") as psum2_real:
            psum2 = psum1 if psum2_bufs == 0 else psum2_real

            # Block-0 is DMA-bound: the first psum groups wait on xgt + w1.
            # `head` picks the DMA issue schedule for the block-0 inputs:
            #   plain     — whole-tensor DMAs, xgt then (w1, w2 inside stage1)
            #   staircase — xgt chunk 0 first, then w1 column tiles / xgt
            #               chunks / w2-block-0 interleaved per the schedule,
            #               and block-0 psum groups emitted in arrival order.
            xgt_sb = cpool.tile([P, DK, C], _f32r)
            w1_first = None
            w2_first = None
            arrival = {}
            if head == "plain":
                for dk in range(DK):
                    nc.sync.dma_start(xgt_sb[:, dk], xgt_r[:, dk])
            elif head.startswith("dkmajor"):
                # per-dk interleave of xgt and w1-block0 (same large-DMA
                # granularity as plain); block-0 stage1 consumes dk-major
                w1_first = w1pool.tile([P, DK, FB], _f32r, tag="w1_t")
                if head == "dkmajor_w2mid":
                    w2_first = w2pool.tile([P, FC, D], _f32r, tag="w2_t")
                for dk in range(DK):
                    nc.sync.dma_start(xgt_sb[:, dk], xgt_r[:, dk])
                    nc.sync.dma_start(w1_first[:, dk], w1_r[:, dk, 0:FB])
                    if head == "dkmajor_w2mid" and 4 <= dk < 4 + FC:
                        fk = dk - 4
                        nc.sync.dma_start(w2_first[:, fk], w2_r[:, fk, :])
            else:
                w1_first = w1pool.tile([P, DK, FB], _f32r, tag="w1_t")
                w2_first = w2pool.tile([P, FC, D], _f32r, tag="w2_t")
                if head == "staircase":
                    issue = ["x0", "w0", "w1", "x1", "w2", "w3", "x2", "W"]
                elif head == "stair_w2early":
                    issue = ["x0", "w0", "w1", "W", "x1", "w2", "w3", "x2"]
                elif head == "stair_w2mid":
                    issue = ["x0", "w0", "w1", "x1", "w2", "w3", "W", "x2"]
                else:
                    raise ValueError(head)
                issue = [t for t in issue
                         if t[0] != "x" or int(t[1:]) < len(chunks)]
                arrival = {tag: i for i, tag in enumerate(issue)}
                for tag in issue:
                    if tag == "W":
                        for fk in range(FC):
                            nc.sync.dma_start(w2_first[:, fk], w2_r[:, fk, :])
                    elif tag[0] == "x":
                        i = int(tag[1:])
                        cs, csz = chunks[i]
                        for dk in range(DK):
                            nc.sync.dma_start(xgt_sb[:, dk, cs:cs + csz],
                                              xgt_r[:, dk, cs:cs + csz])
                    else:
                        i = int(tag[1:])
                        for dk in range(DK):
                            nc.sync.dma_start(w1_first[:, dk, i * P:(i + 1) * P],
                                              w1_r[:, dk, i * P:(i + 1) * P])
            b1_sb = cpool.tile([P, F // P], _f32)
            nc.sync.dma_start(b1_sb[:], b1_d.rearrange("(o p) -> p o", p=P))
            probs_sb = cpool.tile([P, TT], _f32)
            nc.sync.dma_start(probs_sb[:], probs_d.rearrange("(o p) -> p o", p=P))
            y_acc = cpool.tile([P, TT, D], _f32)

            def stage1(fb, first_block=False):
                """load w1/w2 block, produce hT = relu(w1.T @ x + b1); returns (hT, w2_t)."""
                special = first_block and head != "plain"
                if special:
                    w1_t = w1_first
                else:
                    w1_t = w1pool.tile([P, DK, FB], _f32r)
                    for dk in range(DK):
                        nc.sync.dma_start(w1_t[:, dk], w1_r[:, dk, fb * FB:(fb + 1) * FB])
                if special and w2_first is not None:
                    w2_t = w2_first
                elif special:
                    # split per (fk, dn) so stage2(0)'s dn-outer groups can
                    # start once the dn=0 half has landed
                    w2_t = w2pool.tile([P, FC, D], _f32r, tag="w2_t")
                    for dn in range(DN):
                        for fk in range(FC):
                            nc.sync.dma_start(w2_t[:, fk, dn * 512:(dn + 1) * 512],
                                              w2_r[:, fb * FC + fk, dn * 512:(dn + 1) * 512])
                else:
                    w2_t = w2pool.tile([P, FC, D], _f32r)
                    for fk in range(FC):
                        nc.sync.dma_start(w2_t[:, fk], w2_r[:, fb * FC + fk, :])
                hT = hpool.tile([P, FC, C], _f32r)
                groups = [(fc, ci) for fc in range(FC) for ci in range(len(chunks))]
                if special and not head.startswith("dkmajor"):
                    groups.sort(key=lambda g: (max(arrival[f"w{g[0]}"], arrival[f"x{g[1]}"]),
                                               arrival[f"x{g[1]}"], g[0]))

                def relu_group(fc, ci, ps):
                    cs, csz = chunks[ci]
                    fcol = fb * FC + fc
                    nc.scalar.activation(
                        hT[:, fc, cs:cs + csz], ps[:, :csz],
                        mybir.ActivationFunctionType.Relu,
                        bias=b1_sb[:, fcol:fcol + 1], scale=1.0,
                    )

                if special and head.startswith("dkmajor"):
                    # dk-major waves: up to psum1_bufs groups accumulate
                    # concurrently so PE consumes each xgt[dk] as it lands
                    wave_size = psum1_bufs
                    for ws in range(0, len(groups), wave_size):
                        wave = groups[ws:ws + wave_size]
                        pss = [psum1.tile([P, 512], _f32, name=f"ps_w{ws}_{i}", tag="ps")
                               for i in range(len(wave))]
                        for dk in range(DK):
                            for (fc, ci), ps in zip(wave, pss):
                                cs, csz = chunks[ci]
                                nc.tensor.matmul(
                                    ps[:, :csz],
                                    w1_t[:, dk, fc * P:(fc + 1) * P],
                                    xgt_sb[:, dk, cs:cs + csz],
                                    start=(dk == 0), stop=(dk == DK - 1),
                                )
                        for (fc, ci), ps in zip(wave, pss):
                            relu_group(fc, ci, ps)
                else:
                    for (fc, ci) in groups:
                        cs, csz = chunks[ci]
                        ps = psum1.tile([P, 512], _f32)
                        for dk in range(DK):
                            nc.tensor.matmul(
                                ps[:, :csz],
                                w1_t[:, dk, fc * P:(fc + 1) * P],
                                xgt_sb[:, dk, cs:cs + csz],
                                start=(dk == 0), stop=(dk == DK - 1),
                            )
                        relu_group(fc, ci, ps)
                return hT, w2_t

            def stage2(fb, hT, w2_t, first, last):
                """y_acc (+)= hT.T @ w2; on the last block scale by probs and store."""
                if first:
                    # dn-outer: the dn=0 half of w2-block0 lands first
                    s2_groups = [(tc_i, dn) for dn in range(DN) for tc_i in range(TT)]
                else:
                    s2_groups = [(tc_i, dn) for tc_i in range(TT) for dn in range(DN)]
                for (tc_i, dn) in s2_groups:
                    ps2 = psum2.tile([P, 512], _f32,
                                     tag="ps" if psum2_bufs == 0 else "ps2")
                    for fk in range(FC):
                        nc.tensor.matmul(
                            ps2[:],
                            hT[:, fk, tc_i * P:(tc_i + 1) * P],
                            w2_t[:, fk, dn * 512:(dn + 1) * 512],
                            start=(fk == 0), stop=(fk == FC - 1),
                        )
                    ya = y_acc[:, tc_i, dn * 512:(dn + 1) * 512]
                    if first:
                        nc.vector.tensor_copy(ya, ps2[:])
                    else:
                        nc.vector.tensor_add(ya, ps2[:], ya)
                    if last:
                        nc.scalar.activation(
                            ya, ya, mybir.ActivationFunctionType.Copy,
                            scale=probs_sb[:, tc_i:tc_i + 1],
                        )
                        nc.sync.dma_start(y_r[:, tc_i, dn * 512:(dn + 1) * 512], ya)

            n_blocks = passes * NB
            tiles = {}

            def emit_s1(rep):
                fb = rep % NB
                tiles[rep] = (fb, *stage1(fb, first_block=(rep == 0)))

            def emit_s2(rep):
                fb, hT, w2_t = tiles.pop(rep)
                stage2(fb, hT, w2_t, fb == 0, fb == NB - 1)

            if skew and n_blocks >= 2:
                # Block 0 is DMA-bound: emit s2(0) right after s1(0) (PE is
                # waiting on DMA there anyway, the relu bubble is free), then
                # skew by one block so later relus hide under the next s1.
                emit_s1(0)
                emit_s2(0)
                emit_s1(1)
                for rep in range(2, n_blocks):
                    emit_s1(rep)
                    emit_s2(rep - 1)
                emit_s2(n_blocks - 1)
            else:
                for rep in range(n_blocks):
                    emit_s1(rep)
                    emit_s2(rep)
    nc.compile()
    _NC_CACHE[key] = nc
    return nc


class _Runner:
    """Persistent jitted SPMD executor for a compiled Bacc program.

    Mirrors bass2jax.run_bass_via_pjrt but keeps the jitted callable so
    repeat calls skip retracing/recompiling.
    """

    def __init__(self, nc, n_cores):
        import jax
        from jax.sharding import Mesh, PartitionSpec
        from jax.experimental.shard_map import shard_map

        bass2jax.install_neuronx_cc_hook()
        self.nc = nc
        self.n_cores = n_cores
        in_names, out_names, out_avals = [], [], []
        for alloc in nc.m.functions[0].allocations:
            if not isinstance(alloc, _mybir.MemoryLocationSet):
                continue
            name = alloc.memorylocations[0].name
            if alloc.kind == "ExternalInput":
                in_names.append(name)
            elif alloc.kind == "ExternalOutput":
                out_names.append(name)
                out_avals.append(jax.core.ShapedArray(
                    tuple(alloc.tensor_shape), _mybir.dt.np(alloc.dtype)))
        partition_name = nc.partition_id_tensor.name if nc.partition_id_tensor else None
        in_names = [n for n in in_names if n != partition_name]
        all_names = in_names + out_names + ([partition_name] if partition_name else [])
        self.in_names, self.out_names, self.out_avals = in_names, out_names, out_avals
        self._all_names, self._partition_name = all_names, partition_name
        n_params = len(in_names)

        def _body(*args):
            operands = list(args)
            if partition_name is not None:
                operands.append(bass2jax.partition_id_tensor())
            outs = bass2jax._bass_exec_p.bind(
                *operands,
                out_avals=tuple(out_avals),
                in_names=tuple(all_names),
                out_names=tuple(out_names),
                lowering_input_output_aliases=(),
                sim_require_finite=False,
                sim_require_nnan=False,
                nc=nc,
            )
            return tuple(outs)

        devices = jax.devices()[:n_cores]
        mesh = Mesh(np.asarray(devices), ("core",))
        n_outs = len(out_names)
        self._fn = jax.jit(
            shard_map(_body, mesh=mesh,
                      in_specs=(PartitionSpec("core"),) * (n_params + n_outs),
                      out_specs=(PartitionSpec("core"),) * n_outs,
                      check_rep=False),
            donate_argnums=tuple(range(n_params, n_params + n_outs)),
            keep_unused=True,
        )
        self._jax = jax

    def concat_inputs(self, in_maps):
        return [np.concatenate([np.asarray(m[name]) for m in in_maps], axis=0)
                for name in self.in_names]

    def zero_outs(self):
        jnp = self._jax.numpy
        return [jnp.zeros((self.n_cores * a.shape[0], *a.shape[1:]), a.dtype)
                for a in self.out_avals]

    def run_raw(self, concat_in, zouts):
        outs = self._fn(*concat_in, *zouts)
        self._jax.block_until_ready(outs)
        return outs

    def run(self, in_maps):
        outs = self.run_raw(self.concat_inputs(in_maps), self.zero_outs())
        return [
            {name: np.asarray(outs[i]).reshape(self.n_cores, *self.out_avals[i].shape)[c]
             for i, name in enumerate(self.out_names)}
            for c in range(self.n_cores)
        ]


_RUNNER_CACHE: dict = {}


def _runner(C, passes=1):
    key = (C, passes)
    if key not in _RUNNER_CACHE:
        _RUNNER_CACHE[key] = _Runner(_build(C, passes), N_EXPERTS)
    return _RUNNER_CACHE[key]


def _route(x2d, gate_w, gate_b):
    """Host gate: returns per-token top-2 expert ids and softmax probs (fp32)."""
    logits = x2d.astype(np.float64) @ gate_w.astype(np.float64) + gate_b.astype(np.float64)
    order = np.argsort(-logits, axis=-1, kind="stable")
    top2 = order[:, :TOP_K]                               # [T, 2]
    l = np.take_along_axis(logits, top2, axis=-1)         # [T, 2]
    m = l.max(axis=-1, keepdims=True)
    e = np.exp(l - m)
    p = (e / e.sum(axis=-1, keepdims=True)).astype(np.float32)
    return top2, p


def kernel(x, gate_w, gate_b, w1, b1, w2, b2):
    x = np.asarray(x, dtype=np.float32)
    gate_w = np.asarray(gate_w, dtype=np.float32)
    gate_b = np.asarray(gate_b, dtype=np.float32)
    w1 = np.asarray(w1, dtype=np.float32)
    b1 = np.asarray(b1, dtype=np.float32)
    w2 = np.asarray(w2, dtype=np.float32)
    b2 = np.asarray(b2, dtype=np.float32)

    T = S * B
    x2d = np.ascontiguousarray(x.reshape(T, D))
    top2, p = _route(x2d, gate_w, gate_b)

    # dispatch: token lists per expert
    idx_lists = []
    for e in range(N_EXPERTS):
        sel = np.nonzero(top2 == e)          # (token_idx, slot_idx)
        idx_lists.append((sel[0], p[sel[0], sel[1]]))
    max_n = max(len(ix) for ix, _ in idx_lists)

    # capacity cap (SBUF budget): if wildly imbalanced, run multiple passes
    n_pass = max(1, -(-max_n // _C_MAX))
    per_pass = -(-max_n // n_pass)
    C = max(256, -(-per_pass // P) * P)

    global LAST_C
    LAST_C = C
    runner = _runner(C)

    out2d = np.zeros((T, D), dtype=np.float32)
    xT = x2d.T  # [D, T]
    for ps in range(n_pass):
        in_maps = []
        metas = []
        for e in range(N_EXPERTS):
            ix_all, pe_all = idx_lists[e]
            ix = ix_all[ps * C:(ps + 1) * C]
            pe = pe_all[ps * C:(ps + 1) * C]
            n = len(ix)
            xgt = np.zeros((D, C), dtype=np.float32)
            if n:
                xgt[:, :n] = xT[:, ix]
            probs = np.zeros((C,), dtype=np.float32)
            probs[:n] = pe
            in_maps.append({
                "xgt": xgt,
                "w1": np.ascontiguousarray(w1[e]),
                "b1": np.ascontiguousarray(b1[e]),
                "w2": np.ascontiguousarray(w2[e]),
                "probs": probs,
            })
            metas.append((ix, n))
        import time as _time
        _t0 = _time.time()
        results = runner.run(in_maps)
        global LAST_DEVICE_NS
        LAST_DEVICE_NS = int((_time.time() - _t0) * 1e9)
        for e in range(N_EXPERTS):
            ix, n = metas[e]
            if n:
                out2d[ix] += results[e]["y"][:n]  # ix unique per expert

    if np.any(b2):
        comb = np.zeros((T, N_EXPERTS), dtype=np.float32)
        np.put_along_axis(comb, top2, p, axis=-1)
        out2d += comb @ b2
    return out2d.reshape(S, B, D)


# revision 74
# speedup vs baseline: 1.3148x; 1.3148x over previous
"""MoE layer (8 experts, top-2) on 8 TRN2 NeuronCores — expert parallelism.

Contract: kernel(**inputs) takes FULL inputs, returns FULL output.
Strategy:
  - Host computes the (tiny) gate: logits -> top-2 -> softmax. This is the
    dispatch step of expert parallelism: tokens are gathered per expert.
  - Core e gets expert e's weights and its gathered tokens (padded to a
    fixed capacity C), computes y = relu(x @ w1 + b1) @ w2 scaled by the
    gate prob, via a Bass/Tile kernel using float32r matmuls.
  - Host scatter-adds the two expert contributions per token (combine step)
    plus the (usually zero) b2 term.

Shapes (hardcoded from the problem spec):
  x [2048, 2, 1024], gate_w [1024, 8], gate_b [8],
  w1 [8, 1024, 4096], b1 [8, 4096], w2 [8, 4096, 1024], b2 [8, 1024].
"""
import sys
import numpy as np

for _p in ("/opt/trn_rl_repo", "/root/.axon_site/_ro/trn_rl_repo"):
    if _p not in sys.path:
        sys.path.insert(0, _p)

import concourse.bacc as bacc
import concourse.tile as tile
import concourse.mybir as mybir
from concourse import bass2jax, mybir as _mybir

N_EXPERTS = 8
TOP_K = 2
S, B, D, F = 2048, 2, 1024, 4096
P = 128
FB = 512                # F-block size streamed through SBUF
NB = F // FB            # 8 F-blocks
FC = FB // P            # 4 partition-tiles of F per block
DK = D // P             # 8 contraction tiles for stage 1
DN = D // 512           # 2 output-column chunks for stage 2

_f32 = mybir.dt.float32
_f32r = mybir.dt.float32r

_NC_CACHE: dict = {}
_C_MAX = 1280           # max capacity per pass (SBUF budget bound)
LAST_DEVICE_NS = -1     # wall-clock of the last device dispatch (incl. transfers)
LAST_C = -1


def _c_chunks(C):
    """Split C (multiple of 128) into near-equal chunks <=512 and >=256.

    f32r matmuls run at full rate only for moving dim >=256; chunks >=384
    also keep the implicit LDWEIGHTS (~128 cols) hidden under streaming.
    """
    k = -(-C // 512)
    base = (C // k) // P * P
    sizes = [base] * k
    rem = C - base * k
    i = 0
    while rem > 0:
        sizes[i] += P
        rem -= P
        i = (i + 1) % k
    out, pos = [], 0
    for s in sizes:
        out.append((pos, s))
        pos += s
    return out


def _build(C, passes=1, *, skew=True, psum1_bufs=8, psum2_bufs=0, w_bufs=2, h_bufs=2,
           w1_bufs=None, head="dkmajor"):
    """Trace + compile the per-core SPMD program for capacity C (multiple of 128, >=256).

    passes>1 repeats the whole compute (same output) — used only for
    differential timing of the device kernel.
    skew=True emits stage1 of block fb+1 before stage2 of block fb so the
    PE can keep streaming matmuls while ACT finishes the relus stage2 needs.
    """
    if w1_bufs is None:
        w1_bufs = w_bufs
    key = (C, passes, skew, psum1_bufs, psum2_bufs, w_bufs, h_bufs, w1_bufs, head)
    if key in _NC_CACHE:
        return _NC_CACHE[key]
    TT = C // P
    nc = bacc.Bacc("TRN2", target_bir_lowering=False, debug=False,
                   enable_asserts=False, num_devices=8)
    xgt_d = nc.dram_tensor("xgt", (D, C), _f32r, kind="ExternalInput").ap()
    w1_d = nc.dram_tensor("w1", (D, F), _f32r, kind="ExternalInput").ap()
    b1_d = nc.dram_tensor("b1", (F,), _f32, kind="ExternalInput").ap()
    w2_d = nc.dram_tensor("w2", (F, D), _f32r, kind="ExternalInput").ap()
    probs_d = nc.dram_tensor("probs", (C,), _f32, kind="ExternalInput").ap()
    y_d = nc.dram_tensor("y", (C, D), _f32, kind="ExternalOutput").ap()

    xgt_r = xgt_d.rearrange("(ko ki) c -> ki ko c", ki=P)      # [128, 8, C]
    w1_r = w1_d.rearrange("(ko ki) f -> ki ko f", ki=P)        # [128, 8, F]
    w2_r = w2_d.rearrange("(fo fi) d -> fi fo d", fi=P)        # [128, 32, D]
    y_r = y_d.rearrange("(t p) d -> p t d", p=P)               # [128, TT, D]

    chunks = _c_chunks(C)

    with tile.TileContext(nc) as tc:
        with tc.tile_pool(name="const", bufs=1) as cpool, \
             tc.tile_pool(name="w1p", bufs=w1_bufs) as w1pool, \
             tc.tile_pool(name="w2p", bufs=w_bufs) as w2pool, \
             tc.tile_pool(name="hp", bufs=h_bufs) as hpool, \
             tc.tile_pool(name="ps1", bufs=psum1_bufs, space="PSUM") as psum1, \
             tc.tile_pool(name="ps2", bufs=max(psum2_bufs, 1), space="PSUM") as psum2_real:
            psum2 = psum1 if psum2_bufs <= 1 else psum2_real

            # Block-0 is DMA-bound: the first psum groups wait on xgt + w1.
            # `head` picks the DMA issue schedule for the block-0 inputs:
            #   plain     — whole-tensor DMAs, xgt then (w1, w2 inside stage1)
            #   staircase — xgt chunk 0 first, then w1 column tiles / xgt
            #               chunks / w2-block-0 interleaved per the schedule,
            #               and block-0 psum groups emitted in arrival order.
            xgt_sb = cpool.tile([P, DK, C], _f32r)
            w1_first = None
            w2_first = None
            arrival = {}
            if head == "plain":
                for dk in range(DK):
                    nc.sync.dma_start(xgt_sb[:, dk], xgt_r[:, dk])
            elif head == "dkmajor3":
                # chunk-0 slices of xgt front-loaded (with w1-block0
                # interleaved), remaining chunks streamed after; block-0
                # stage1 runs per-chunk dk-major waves
                w1_first = w1pool.tile([P, DK, FB], _f32r, tag="w1_t")
                cs0, csz0 = chunks[0]
                for dk in range(DK):
                    nc.sync.dma_start(xgt_sb[:, dk, cs0:cs0 + csz0],
                                      xgt_r[:, dk, cs0:cs0 + csz0])
                    nc.sync.dma_start(w1_first[:, dk], w1_r[:, dk, 0:FB])
                if len(chunks) > 1:
                    cs1 = chunks[1][0]
                    for dk in range(DK):
                        nc.sync.dma_start(xgt_sb[:, dk, cs1:], xgt_r[:, dk, cs1:])
            elif head.startswith("dkmajor"):
                # per-dk interleave of xgt and w1-block0 (same large-DMA
                # granularity as plain); block-0 stage1 consumes dk-major
                w1_first = w1pool.tile([P, DK, FB], _f32r, tag="w1_t")
                if head == "dkmajor_w2mid":
                    w2_first = w2pool.tile([P, FC, D], _f32r, tag="w2_t")
                for dk in range(DK):
                    nc.sync.dma_start(xgt_sb[:, dk], xgt_r[:, dk])
                    nc.sync.dma_start(w1_first[:, dk], w1_r[:, dk, 0:FB])
                    if head == "dkmajor_w2mid" and 4 <= dk < 4 + FC:
                        fk = dk - 4
                        nc.sync.dma_start(w2_first[:, fk], w2_r[:, fk, :])
            else:
                w1_first = w1pool.tile([P, DK, FB], _f32r, tag="w1_t")
                w2_first = w2pool.tile([P, FC, D], _f32r, tag="w2_t")
                if head == "staircase":
                    issue = ["x0", "w0", "w1", "x1", "w2", "w3", "x2", "W"]
                elif head == "stair_w2early":
                    issue = ["x0", "w0", "w1", "W", "x1", "w2", "w3", "x2"]
                elif head == "stair_w2mid":
                    issue = ["x0", "w0", "w1", "x1", "w2", "w3", "W", "x2"]
                else:
                    raise ValueError(head)
                issue = [t for t in issue
                         if t[0] != "x" or int(t[1:]) < len(chunks)]
                arrival = {tag: i for i, tag in enumerate(issue)}
                for tag in issue:
                    if tag == "W":
                        for fk in range(FC):
                            nc.sync.dma_start(w2_first[:, fk], w2_r[:, fk, :])
                    elif tag[0] == "x":
                        i = int(tag[1:])
                        cs, csz = chunks[i]
                        for dk in range(DK):
                            nc.sync.dma_start(xgt_sb[:, dk, cs:cs + csz],
                                              xgt_r[:, dk, cs:cs + csz])
                    else:
                        i = int(tag[1:])
                        for dk in range(DK):
                            nc.sync.dma_start(w1_first[:, dk, i * P:(i + 1) * P],
                                              w1_r[:, dk, i * P:(i + 1) * P])
            b1_sb = cpool.tile([P, F // P], _f32)
            nc.sync.dma_start(b1_sb[:], b1_d.rearrange("(o p) -> p o", p=P))
            probs_sb = cpool.tile([P, TT], _f32)
            nc.sync.dma_start(probs_sb[:], probs_d.rearrange("(o p) -> p o", p=P))
            y_acc = cpool.tile([P, TT, D], _f32)

            def stage1(fb, first_block=False):
                """load w1/w2 block, produce hT = relu(w1.T @ x + b1); returns (hT, w2_t)."""
                special = first_block and head != "plain"
                if special:
                    w1_t = w1_first
                else:
                    w1_t = w1pool.tile([P, DK, FB], _f32r)
                    for dk in range(DK):
                        nc.sync.dma_start(w1_t[:, dk], w1_r[:, dk, fb * FB:(fb + 1) * FB])
                if special and w2_first is not None:
                    w2_t = w2_first
                elif special:
                    # split per (fk, dn) so stage2(0)'s dn-outer groups can
                    # start once the dn=0 half has landed
                    w2_t = w2pool.tile([P, FC, D], _f32r, tag="w2_t")
                    for dn in range(DN):
                        for fk in range(FC):
                            nc.sync.dma_start(w2_t[:, fk, dn * 512:(dn + 1) * 512],
                                              w2_r[:, fb * FC + fk, dn * 512:(dn + 1) * 512])
                else:
                    w2_t = w2pool.tile([P, FC, D], _f32r)
                    for fk in range(FC):
                        nc.sync.dma_start(w2_t[:, fk], w2_r[:, fb * FC + fk, :])
                hT = hpool.tile([P, FC, C], _f32r)
                groups = [(fc, ci) for fc in range(FC) for ci in range(len(chunks))]
                if special and not head.startswith("dkmajor"):
                    groups.sort(key=lambda g: (max(arrival[f"w{g[0]}"], arrival[f"x{g[1]}"]),
                                               arrival[f"x{g[1]}"], g[0]))

                def relu_group(fc, ci, ps, on_dve=False):
                    cs, csz = chunks[ci]
                    fcol = fb * FC + fc
                    if on_dve:
                        # relu(ps + b1) in one DVE op — block-0 only, where
                        # the serial ACT relu trail gates psum slot recycling
                        nc.vector.tensor_scalar(
                            hT[:, fc, cs:cs + csz], ps[:, :csz],
                            b1_sb[:, fcol:fcol + 1], 0.0,
                            mybir.AluOpType.add, mybir.AluOpType.max,
                        )
                    else:
                        nc.scalar.activation(
                            hT[:, fc, cs:cs + csz], ps[:, :csz],
                            mybir.ActivationFunctionType.Relu,
                            bias=b1_sb[:, fcol:fcol + 1], scale=1.0,
                        )

                if special and head.startswith("dkmajor"):
                    # dk-major waves: up to psum1_bufs groups accumulate
                    # concurrently so PE consumes each xgt[dk] as it lands
                    # chunk-major so the relus stage2's first groups need
                    # (all fc of chunk 0) complete in wave A, not wave B
                    groups.sort(key=lambda g: (g[1], g[0]))
                    wave_size = FC if head == "dkmajor3" else psum1_bufs
                    for ws in range(0, len(groups), wave_size):
                        wave = groups[ws:ws + wave_size]
                        pss = [psum1.tile([P, 512], _f32, name=f"ps_w{ws}_{i}", tag="ps")
                               for i in range(len(wave))]
                        for dk in range(DK):
                            for (fc, ci), ps in zip(wave, pss):
                                cs, csz = chunks[ci]
                                nc.tensor.matmul(
                                    ps[:, :csz],
                                    w1_t[:, dk, fc * P:(fc + 1) * P],
                                    xgt_sb[:, dk, cs:cs + csz],
                                    start=(dk == 0), stop=(dk == DK - 1),
                                )
                        for i, ((fc, ci), ps) in enumerate(zip(wave, pss)):
                            relu_group(fc, ci, ps, on_dve=(i % 2 == 0))
                else:
                    for (fc, ci) in groups:
                        cs, csz = chunks[ci]
                        ps = psum1.tile([P, 512], _f32)
                        for dk in range(DK):
                            nc.tensor.matmul(
                                ps[:, :csz],
                                w1_t[:, dk, fc * P:(fc + 1) * P],
                                xgt_sb[:, dk, cs:cs + csz],
                                start=(dk == 0), stop=(dk == DK - 1),
                            )
                        relu_group(fc, ci, ps)
                return hT, w2_t

            def stage2(fb, hT, w2_t, first, last):
                """y_acc (+)= hT.T @ w2; on the last block scale by probs and store."""
                if first:
                    # dn-outer: the dn=0 half of w2-block0 lands first
                    s2_groups = [(tc_i, dn) for dn in range(DN) for tc_i in range(TT)]
                else:
                    s2_groups = [(tc_i, dn) for tc_i in range(TT) for dn in range(DN)]
                for gi, (tc_i, dn) in enumerate(s2_groups):
                    if first and gi < 2 and psum2_bufs == 1:
                        # block-0 head groups take the dedicated bank — no
                        # wait on a stage-1 slot release
                        ps2 = psum2_real.tile([P, 512], _f32, name=f"ps2d{gi}", tag="ps2d")
                    elif psum2_bufs <= 1:
                        ps2 = psum2.tile([P, 512], _f32, tag="ps", name=f"ps2_{fb}_{gi}")
                    else:
                        ps2 = psum2.tile([P, 512], _f32, tag="ps2")
                    for fk in range(FC):
                        nc.tensor.matmul(
                            ps2[:],
                            hT[:, fk, tc_i * P:(tc_i + 1) * P],
                            w2_t[:, fk, dn * 512:(dn + 1) * 512],
                            start=(fk == 0), stop=(fk == FC - 1),
                        )
                    ya = y_acc[:, tc_i, dn * 512:(dn + 1) * 512]
                    if first:
                        nc.vector.tensor_copy(ya, ps2[:])
                    else:
                        nc.vector.tensor_add(ya, ps2[:], ya)
                    if last:
                        nc.scalar.activation(
                            ya, ya, mybir.ActivationFunctionType.Copy,
                            scale=probs_sb[:, tc_i:tc_i + 1],
                        )
                        nc.sync.dma_start(y_r[:, tc_i, dn * 512:(dn + 1) * 512], ya)

            n_blocks = passes * NB
            tiles = {}

            def emit_s1(rep):
                fb = rep % NB
                tiles[rep] = (fb, *stage1(fb, first_block=(rep == 0)))

            def emit_s2(rep):
                fb, hT, w2_t = tiles.pop(rep)
                stage2(fb, hT, w2_t, fb == 0, fb == NB - 1)

            if skew and n_blocks >= 2:
                # Block 0 is DMA-bound: emit s2(0) right after s1(0) (PE is
                # waiting on DMA there anyway, the relu bubble is free), then
                # skew by one block so later relus hide under the next s1.
                emit_s1(0)
                emit_s2(0)
                emit_s1(1)
                for rep in range(2, n_blocks):
                    emit_s1(rep)
                    emit_s2(rep - 1)
                emit_s2(n_blocks - 1)
            else:
                for rep in range(n_blocks):
                    emit_s1(rep)
                    emit_s2(rep)
    nc.compile()
    _NC_CACHE[key] = nc
    return nc


class _Runner:
    """Persistent jitted SPMD executor for a compiled Bacc program.

    Mirrors bass2jax.run_bass_via_pjrt but keeps the jitted callable so
    repeat calls skip retracing/recompiling.
    """

    def __init__(self, nc, n_cores):
        import jax
        from jax.sharding import Mesh, PartitionSpec
        from jax.experimental.shard_map import shard_map

        bass2jax.install_neuronx_cc_hook()
        self.nc = nc
        self.n_cores = n_cores
        in_names, out_names, out_avals = [], [], []
        for alloc in nc.m.functions[0].allocations:
            if not isinstance(alloc, _mybir.MemoryLocationSet):
                continue
            name = alloc.memorylocations[0].name
            if alloc.kind == "ExternalInput":
                in_names.append(name)
            elif alloc.kind == "ExternalOutput":
                out_names.append(name)
                out_avals.append(jax.core.ShapedArray(
                    tuple(alloc.tensor_shape), _mybir.dt.np(alloc.dtype)))
        partition_name = nc.partition_id_tensor.name if nc.partition_id_tensor else None
        in_names = [n for n in in_names if n != partition_name]
        all_names = in_names + out_names + ([partition_name] if partition_name else [])
        self.in_names, self.out_names, self.out_avals = in_names, out_names, out_avals
        self._all_names, self._partition_name = all_names, partition_name
        n_params = len(in_names)

        def _body(*args):
            operands = list(args)
            if partition_name is not None:
                operands.append(bass2jax.partition_id_tensor())
            outs = bass2jax._bass_exec_p.bind(
                *operands,
                out_avals=tuple(out_avals),
                in_names=tuple(all_names),
                out_names=tuple(out_names),
                lowering_input_output_aliases=(),
                sim_require_finite=False,
                sim_require_nnan=False,
                nc=nc,
            )
            return tuple(outs)

        devices = jax.devices()[:n_cores]
        mesh = Mesh(np.asarray(devices), ("core",))
        n_outs = len(out_names)
        self._fn = jax.jit(
            shard_map(_body, mesh=mesh,
                      in_specs=(PartitionSpec("core"),) * (n_params + n_outs),
                      out_specs=(PartitionSpec("core"),) * n_outs,
                      check_rep=False),
            donate_argnums=tuple(range(n_params, n_params + n_outs)),
            keep_unused=True,
        )
        self._jax = jax

    def concat_inputs(self, in_maps):
        return [np.concatenate([np.asarray(m[name]) for m in in_maps], axis=0)
                for name in self.in_names]

    def zero_outs(self):
        jnp = self._jax.numpy
        return [jnp.zeros((self.n_cores * a.shape[0], *a.shape[1:]), a.dtype)
                for a in self.out_avals]

    def run_raw(self, concat_in, zouts):
        outs = self._fn(*concat_in, *zouts)
        self._jax.block_until_ready(outs)
        return outs

    def run(self, in_maps):
        outs = self.run_raw(self.concat_inputs(in_maps), self.zero_outs())
        return [
            {name: np.asarray(outs[i]).reshape(self.n_cores, *self.out_avals[i].shape)[c]
             for i, name in enumerate(self.out_names)}
            for c in range(self.n_cores)
        ]


_RUNNER_CACHE: dict = {}


def _runner(C, passes=1):
    key = (C, passes)
    if key not in _RUNNER_CACHE:
        _RUNNER_CACHE[key] = _Runner(_build(C, passes), N_EXPERTS)
    return _RUNNER_CACHE[key]


def _route(x2d, gate_w, gate_b):
    """Host gate: returns per-token top-2 expert ids and softmax probs (fp32)."""
    logits = x2d.astype(np.float64) @ gate_w.astype(np.float64) + gate_b.astype(np.float64)
    order = np.argsort(-logits, axis=-1, kind="stable")
    top2 = order[:, :TOP_K]                               # [T, 2]
    l = np.take_along_axis(logits, top2, axis=-1)         # [T, 2]
    m = l.max(axis=-1, keepdims=True)
    e = np.exp(l - m)
    p = (e / e.sum(axis=-1, keepdims=True)).astype(np.float32)
    return top2, p


def kernel(x, gate_w, gate_b, w1, b1, w2, b2):
    x = np.asarray(x, dtype=np.float32)
    gate_w = np.asarray(gate_w, dtype=np.float32)
    gate_b = np.asarray(gate_b, dtype=np.float32)
    w1 = np.asarray(w1, dtype=np.float32)
    b1 = np.asarray(b1, dtype=np.float32)
    w2 = np.asarray(w2, dtype=np.float32)
    b2 = np.asarray(b2, dtype=np.float32)

    T = S * B
    x2d = np.ascontiguousarray(x.reshape(T, D))
    top2, p = _route(x2d, gate_w, gate_b)

    # dispatch: token lists per expert
    idx_lists = []
    for e in range(N_EXPERTS):
        sel = np.nonzero(top2 == e)          # (token_idx, slot_idx)
        idx_lists.append((sel[0], p[sel[0], sel[1]]))
    max_n = max(len(ix) for ix, _ in idx_lists)

    # capacity cap (SBUF budget): if wildly imbalanced, run multiple passes
    n_pass = max(1, -(-max_n // _C_MAX))
    per_pass = -(-max_n // n_pass)
    C = max(256, -(-per_pass // P) * P)

    global LAST_C
    LAST_C = C
    runner = _runner(C)

    out2d = np.zeros((T, D), dtype=np.float32)
    xT = x2d.T  # [D, T]
    for ps in range(n_pass):
        in_maps = []
        metas = []
        for e in range(N_EXPERTS):
            ix_all, pe_all = idx_lists[e]
            ix = ix_all[ps * C:(ps + 1) * C]
            pe = pe_all[ps * C:(ps + 1) * C]
            n = len(ix)
            xgt = np.zeros((D, C), dtype=np.float32)
            if n:
                xgt[:, :n] = xT[:, ix]
            probs = np.zeros((C,), dtype=np.float32)
            probs[:n] = pe
            in_maps.append({
                "xgt": xgt,
                "w1": np.ascontiguousarray(w1[e]),
                "b1": np.ascontiguousarray(b1[e]),
                "w2": np.ascontiguousarray(w2[e]),
                "probs": probs,
            })
            metas.append((ix, n))
        import time as _time
        _t0 = _time.time()
        results = runner.run(in_maps)
        global LAST_DEVICE_NS
        LAST_DEVICE_NS = int((_time.time() - _t0) * 1e9)
        for e in range(N_EXPERTS):
            ix, n = metas[e]
            if n:
                out2d[ix] += results[e]["y"][:n]  # ix unique per expert

    if np.any(b2):
        comb = np.zeros((T, N_EXPERTS), dtype=np.float32)
        np.put_along_axis(comb, top2, p, axis=-1)
        out2d += comb @ b2
    return out2d.reshape(S, B, D)
